# revision 10
# baseline (speedup 1.0000x reference)
"""Fused 3-layer GAT + global-mean-pool + MLP for Trainium2, 8 NeuronCores.

Design (single fused SPMD program, destination-sharded):
- Per layer, a node table [N, 128] fp16 (cols 0:32 h0 feats, 32 ones,
  33:65 h1 feats, 65 ones, 66:68 alpha_src per head) is built on device
  from the local x slice (x @ Wcat, Wcat host-folded with attention
  vectors) and AllGathered to every core's DRAM.
- Edges are partitioned by destination pair (128 consecutive local dst
  nodes); per-edge source rows are fetched with dma_gather (int16
  indices local to four 25000-node source regions, <=1024 idx/instr).
- alpha_dst per edge via one-hot matmul (host-precomputed u8 transposed
  one-hots); leaky-relu+exp on DVE/ACT; segment softmax folded into a
  ratio of one-hot segment-sum matmuls (denominator = the ones column
  scaled by exp like everything else).
- ELU epilogue per pair; layers 0/1 write xn pair tiles to DRAM; layer 2
  feeds the global-mean-pool matmul directly; tiny MLP; AllReduce.
"""
import sys
sys.path.insert(0, '/opt/trn_rl_repo')
import numpy as np
import concourse.bass as bass
import concourse.bacc as bacc
import concourse.mybir as mybir
from concourse import library_config
from concourse.tile import TileContext
from concourse.bass_utils import run_bass_kernel_spmd

P = 128
NREG = 4
REGSZ = 25000
ES = 128                 # gather row: 128 fp16 = 256B
MAXG = 8                 # max groups (of 128 idx) per dma_gather (1024 idx cap)
NEG = 0.2
F16 = mybir.dt.float16
F32 = mybir.dt.float32
U8 = mybir.dt.uint8
I16 = mybir.dt.int16

N, F_IN, H, C, E, G = 100000, 128, 2, 32, 3200000, 64
HC = H * C               # 64
NCORE = 8
NL = N // NCORE          # 12500
NPAIR = (NL + P - 1) // P  # 98
CPC = 2                  # pairs per chunk
NCHUNK = NPAIR // CPC    # 49


def _wcat(W, a_src, a_dst):
    """[fin, 70] fp16: feature cols + ones slots(0) + alpha coefficient cols."""
    W = np.asarray(W, np.float32)          # [HC, fin]
    fin = W.shape[1]
    a_s = np.asarray(a_src, np.float32).reshape(H, C)
    a_d = np.asarray(a_dst, np.float32).reshape(H, C)
    out = np.zeros((fin, 70), np.float32)
    out[:, 0:32] = W[0:32, :].T
    out[:, 33:65] = W[32:64, :].T
    for h in range(H):
        out[:, 66 + h] = W[h * C:(h + 1) * C, :].T @ a_s[h]
        out[:, 68 + h] = W[h * C:(h + 1) * C, :].T @ a_d[h]
    return out.astype(np.float16)


def prep(inputs):
    x = np.asarray(inputs['x'], np.float32)
    ei = np.asarray(inputs['edge_index']).astype(np.int64)
    batch = np.asarray(inputs['batch']).astype(np.int64)
    loops = np.arange(N, dtype=np.int64)
    src = np.concatenate([ei[0], loops])
    dst = np.concatenate([ei[1], loops])

    core = dst // NL
    dloc = dst - core * NL
    pair = dloc >> 7
    rel = dloc & 127
    reg = src // REGSZ
    esl = (src - reg * REGSZ).astype(np.int16)

    # per-core sorted edge arrays
    per_core = []
    cnt = np.zeros((NCORE, NPAIR, NREG), np.int64)
    for c in range(NCORE):
        m = core == c
        key = pair[m] * NREG + reg[m]
        o = np.argsort(key, kind='stable')
        per_core.append((esl[m][o], rel[m][o].astype(np.uint8)))
        cnt[c] = np.bincount(key, minlength=NPAIR * NREG).reshape(NPAIR, NREG)

    g = -(-cnt.max(axis=0) // P)          # [NPAIR, NREG] padded group counts
    NG = int(g.sum())

    # chunk metadata (uniform across cores)
    # group layout per chunk k: for r in regions: for j in (0,1): g[2k+j, r] groups
    chunk_meta = []
    goff_global = 0
    soff_global = 0
    for k in range(NCHUNK):
        pair_of_group = []
        instrs = []   # (goff_in_chunk, ngroups, soff_global_cols, region)
        reloff = goff_global
        gc = 0
        for r in range(NREG):
            gcr = int(g[2 * k, r] + g[2 * k + 1, r])
            pair_of_group += [0] * int(g[2 * k, r]) + [1] * int(g[2 * k + 1, r])
            done = 0
            while done < gcr:
                run = min(MAXG, gcr - done)
                instrs.append((gc + done, run, soff_global + done * 8, r))
                done += run
            gc += gcr
            soff_global += gcr * 8
        goff_global += gc
        chunk_meta.append(dict(gc=gc, instrs=instrs, pair_of_group=pair_of_group,
                               goff=reloff))
    assert goff_global == NG

    # per-core padded flat arrays in chunk-group order
    cores = []
    boff = np.zeros((NPAIR, NREG), np.int64)   # group offset of each (p, r) block
    for k in range(NCHUNK):
        off = chunk_meta[k]['goff']
        for r in range(NREG):
            for j in range(CPC):
                boff[2 * k + j, r] = off
                off += int(g[2 * k + j, r])
    for c in range(NCORE):
        es_c, rel_c = per_core[c]
        flat_idx = np.zeros(NG * P, np.int16)
        flat_rel = np.full(NG * P, 255, np.uint8)
        pos = 0
        for p in range(NPAIR):
            for r in range(NREG):
                n = int(cnt[c, p, r])
                b = int(boff[p, r]) * P
                flat_idx[b:b + n] = es_c[pos:pos + n]
                flat_rel[b:b + n] = rel_c[pos:pos + n]
                pos += n
        assert pos == len(es_c)
        idx16 = np.tile(np.ascontiguousarray(flat_idx.reshape(-1, 16).T), (8, 1))
        relmat = np.ascontiguousarray(
            flat_rel.reshape(NG, P).T.astype(np.float16))
        z = np.zeros((NG, P, P), np.uint8)     # [g, d, e]
        v = flat_rel < P
        gi = np.arange(NG * P) // P
        eidx = np.arange(NG * P) % P
        z[gi[v], flat_rel[v].astype(np.int64), eidx[v]] = 1
        oht8 = np.ascontiguousarray(z.transpose(1, 0, 2).reshape(P, NG * P))

        n0 = c * NL
        b_loc = batch[n0:n0 + NL]
        pg = np.zeros((NPAIR * P, G), np.float16)
        pg[np.arange(NL), b_loc] = 1.0
        pg = np.ascontiguousarray(
            pg.reshape(NPAIR, P, G).transpose(1, 0, 2).reshape(P, NPAIR * G))
        cores.append(dict(
            xT16=np.ascontiguousarray(x[n0:n0 + NL].T.astype(np.float16)),
            idx16=np.ascontiguousarray(idx16),
            relmat=relmat,
            oht8=oht8,
            pg=pg,
        ))

    counts = np.bincount(batch, minlength=G).astype(np.float32).reshape(G, 1)
    iota = np.tile(np.arange(P, dtype=np.float16)[None, :], (P, 1))
    id16 = np.eye(P, dtype=np.float16)
    shared = dict(
        NG=NG, chunk_meta=chunk_meta,
        wcat0=_wcat(inputs['W0'], inputs['a_src0'], inputs['a_dst0']),
        wcat1=_wcat(inputs['W1'], inputs['a_src1'], inputs['a_dst1']),
        wcat2=_wcat(inputs['W2'], inputs['a_src2'], inputs['a_dst2']),
        iota=np.ascontiguousarray(iota), id16=np.ascontiguousarray(id16),
        counts=counts,
        w1T=np.ascontiguousarray(np.asarray(inputs['mlp_w1'], np.float32).T
                                 .astype(np.float16)),
        w2T=np.ascontiguousarray(np.asarray(inputs['mlp_w2'], np.float32).T
                                 .astype(np.float16)),
        b1rep=np.tile(np.asarray(inputs['mlp_b1'], np.float32)[None, :], (G, 1)),
        b2rep=np.tile(np.asarray(inputs['mlp_b2'], np.float32)[None, :], (G, 1)),
    )
    for l in range(3):
        assert np.all(np.asarray(inputs[f'b{l}']) == 0.0)
    return cores, shared


def build(shared):
    from contextlib import ExitStack
    NG = shared['NG']
    chunk_meta = shared['chunk_meta']
    AF = mybir.ActivationFunctionType
    OP = mybir.AluOpType

    nc = bacc.Bacc("TRN2", target_bir_lowering=False, debug=False,
                   num_devices=NCORE)
    t_xT = nc.dram_tensor("xT16", [F_IN, NL], F16, kind="ExternalInput")
    t_idx = nc.dram_tensor("idx16", [P, NG * 8], I16, kind="ExternalInput")
    t_rel = nc.dram_tensor("relmat", [P, NG], F16, kind="ExternalInput")
    t_oht = nc.dram_tensor("oht8", [P, NG * P], U8, kind="ExternalInput")
    t_iota = nc.dram_tensor("iota", [P, P], F16, kind="ExternalInput")
    t_id16 = nc.dram_tensor("id16", [P, P], F16, kind="ExternalInput")
    t_wc = [nc.dram_tensor(f"wcat{l}", [F_IN if l == 0 else HC, 70], F16,
                           kind="ExternalInput") for l in range(3)]
    t_pg = nc.dram_tensor("pg", [P, NPAIR * G], F16, kind="ExternalInput")
    t_cnt = nc.dram_tensor("counts", [G, 1], F32, kind="ExternalInput")
    t_w1T = nc.dram_tensor("w1T", [HC, 32], F16, kind="ExternalInput")
    t_w2T = nc.dram_tensor("w2T", [32, 2], F16, kind="ExternalInput")
    t_b1 = nc.dram_tensor("b1rep", [G, 32], F32, kind="ExternalInput")
    t_b2 = nc.dram_tensor("b2rep", [G, 2], F32, kind="ExternalInput")
    t_out = nc.dram_tensor("out", [G, 2], F32, kind="ExternalOutput")

    with TileContext(nc) as tc:
        with ExitStack() as ctx:
            const = ctx.enter_context(tc.tile_pool(name="const", bufs=1))
            sbT = ctx.enter_context(tc.tile_pool(name="sbT", bufs=4))
            sbC = ctx.enter_context(tc.tile_pool(name="sbC", bufs=2))
            sbP = ctx.enter_context(tc.tile_pool(name="sbP", bufs=4))
            psT = ctx.enter_context(tc.tile_pool(name="psT", bufs=1,
                                                 space="PSUM"))
            psA = ctx.enter_context(tc.tile_pool(name="psA", bufs=1,
                                                 space="PSUM"))
            psG = ctx.enter_context(tc.tile_pool(name="psG", bufs=2,
                                                 space="PSUM"))
            psP = ctx.enter_context(tc.tile_pool(name="psP", bufs=1,
                                                 space="PSUM"))
            dramX = ctx.enter_context(tc.tile_pool(name="dramX", bufs=2 * NPAIR + 4,
                                                   space="DRAM"))
            dramT = ctx.enter_context(tc.tile_pool(name="dramT", bufs=2,
                                                   space="DRAM"))
            dramS = ctx.enter_context(tc.tile_pool(name="dramS", bufs=2,
                                                   space="DRAM"))
            dramP = ctx.enter_context(tc.tile_pool(name="dramP", bufs=2,
                                                   space="DRAM"))

            nc.gpsimd.load_library(library_config.mlp)
            iota = const.tile([P, P], F16)
            nc.sync.dma_start(out=iota[:], in_=t_iota.ap())
            id16 = const.tile([P, P], F16)
            nc.sync.dma_start(out=id16[:], in_=t_id16.ap())
            wcs = []
            for l in range(3):
                fin = F_IN if l == 0 else HC
                w = const.tile([fin, 70], F16, tag=f"wc{l}")
                nc.sync.dma_start(out=w[:], in_=t_wc[l].ap())
                wcs.append(w)

            ps_pool = psP.tile([G, HC], F32, space="PSUM")
            xn_tiles = None
            for l in range(3):
                fin = F_IN if l == 0 else HC
                # ---- table build ----
                slice_t = dramS.tile([NL, ES], F16, tag="slice")
                adloc = dramS.tile([NL, 2], F16, tag="adloc")
                for c in range(NPAIR):
                    pc = NL - c * P if c == NPAIR - 1 else P
                    xT = sbT.tile([fin, P], F16, tag="xT")
                    if l == 0:
                        nc.sync.dma_start(out=xT[:, 0:pc],
                                          in_=t_xT.ap()[:, c * P:c * P + pc])
                    else:
                        xc = sbT.tile([P, HC], F16, tag="xc")
                        nc.sync.dma_start(out=xc[:pc], in_=xn_tiles[c][:])
                        ps_t = psT.tile([HC, P], F16, space="PSUM", tag="ps_t")
                        nc.tensor.transpose(out=ps_t[:, 0:pc], in_=xc[:pc],
                                            identity=id16[:pc, :pc])
                        nc.scalar.copy(xT[:, 0:pc], ps_t[:, 0:pc])
                    ps_r = psT.tile([P, 70], F32, space="PSUM", tag="ps_r")
                    nc.tensor.matmul(out=ps_r[:pc], lhsT=xT[:, 0:pc],
                                     rhs=wcs[l][:], start=True, stop=True)
                    tt = sbT.tile([P, 70], F16, tag="tt")
                    nc.scalar.copy(tt[:pc], ps_r[:pc])
                    nc.vector.memset(tt[:pc, 32:33], 1.0)
                    nc.vector.memset(tt[:pc, 65:66], 1.0)
                    nc.sync.dma_start(out=slice_t[c * P:c * P + pc, 0:70],
                                      in_=tt[:pc])
                    nc.sync.dma_start(out=adloc[c * P:c * P + pc, :],
                                      in_=tt[:pc, 68:70])
                table = dramT.tile([N, ES], F16, tag="table")
                nc.gpsimd.collective_compute(
                    "AllGather", OP.bypass,
                    replica_groups=[list(range(NCORE))],
                    ins=[slice_t.opt()], outs=[table.opt()])
                # ---- window chunks ----
                xn_next = [None] * NPAIR
                for k in range(NCHUNK):
                    cm = chunk_meta[k]
                    Gc = cm['gc']
                    pog = cm['pair_of_group']
                    stot = sum(ni * 8 for (_, ni, _, _) in cm['instrs'])
                    s0 = min(so for (_, _, so, _) in cm['instrs'])
                    idxt = sbC.tile([P, stot], I16, tag="idxt")
                    nc.sync.dma_start(out=idxt[:],
                                      in_=t_idx.ap()[:, s0:s0 + stot])
                    relt = sbC.tile([P, Gc], F16, tag="relt")
                    nc.sync.dma_start(
                        out=relt[:],
                        in_=t_rel.ap()[:, cm['goff']:cm['goff'] + Gc])
                    oht = sbC.tile([P, Gc * P], U8, tag="oht")
                    nc.sync.dma_start(
                        out=oht[:],
                        in_=t_oht.ap()[:, cm['goff'] * P:(cm['goff'] + Gc) * P])
                    ohtf = sbC.tile([P, Gc * P], F16, tag="ohtf")
                    nc.vector.tensor_copy(ohtf[:], oht[:])
                    gt = sbC.tile([P, Gc, ES], F16, tag="gt")
                    for (go, ngr, so, r) in cm['instrs']:
                        nidx = ngr * P
                        nc.gpsimd.dma_gather(
                            gt[:, go:go + ngr, :],
                            table[r * REGSZ:, :],
                            idxt[:, so - s0:so - s0 + ngr * 8],
                            nidx, nidx, ES)
                    adws = []
                    for j in range(CPC):
                        p = k * CPC + j
                        pc = NL - p * P if p == NPAIR - 1 else P
                        adw = sbP.tile([P, 2], F16, tag="adw")
                        if pc < P:
                            nc.vector.memset(adw[:], 0.0)
                        nc.sync.dma_start(out=adw[:pc],
                                          in_=adloc[p * P:p * P + pc, :])
                        adws.append((adw, pc))
                    ps_ad = psA.tile([P, 2 * Gc], F32, space="PSUM", tag="ps_ad")
                    for gidx in range(Gc):
                        nc.tensor.matmul(
                            out=ps_ad[:, 2 * gidx:2 * gidx + 2],
                            lhsT=ohtf[:, gidx * P:(gidx + 1) * P],
                            rhs=adws[pog[gidx]][0][:], start=True, stop=True)
                    adh = sbC.tile([P, 2 * Gc], F16, tag="adh")
                    nc.scalar.copy(adh[:], ps_ad[:])
                    ew = sbC.tile([P, 2 * Gc], F16, tag="ew")
                    nc.vector.tensor_tensor(
                        out=ew[:].rearrange("p (g h) -> p g h", h=2),
                        in0=gt[:, :, 66:68],
                        in1=adh[:].rearrange("p (g h) -> p g h", h=2), op=OP.add)
                    ew2 = sbC.tile([P, 2 * Gc], F16, tag="ew2")
                    nc.vector.tensor_scalar_mul(ew2[:], ew[:], NEG)
                    nc.vector.tensor_tensor(out=ew[:], in0=ew[:], in1=ew2[:],
                                            op=OP.max)
                    ewb = sbC.tile([P, 2 * Gc], F16, tag="ewb")
                    nc.scalar.activation(ewb[:], ew[:], AF.Exp)
                    gts = sbC.tile([P, Gc, 66], F16, tag="gts")
                    nc.vector.tensor_tensor(
                        out=gts[:].rearrange("p g (h c) -> p g h c", h=2),
                        in0=gt[:, :, 0:66].rearrange("p g (h c) -> p g h c", h=2),
                        in1=ewb[:].rearrange("p (g h u) -> p g h u", h=2, u=1)
                            .to_broadcast([P, Gc, 2, 33]),
                        op=OP.mult)
                    mall = sbC.tile([P, Gc * P], F16, tag="mall")
                    nc.vector.tensor_tensor(
                        out=mall[:].rearrange("p (g d) -> p g d", g=Gc),
                        in0=iota[:].rearrange("p (u d) -> p u d", u=1)
                            .to_broadcast([P, Gc, P]),
                        in1=relt[:].rearrange("p (g u) -> p g u", u=1)
                            .to_broadcast([P, Gc, P]),
                        op=OP.is_equal)
                    ps_gs = [psG.tile([P, 66], F32, space="PSUM", name=f"ps_g{j}",
                                      tag=f"ps_g{j}") for j in range(CPC)]
                    seen = [False, False]
                    last = [max((i for i, jj in enumerate(pog) if jj == j),
                                default=-1) for j in range(CPC)]
                    for gidx in range(Gc):
                        j = pog[gidx]
                        nc.tensor.matmul(
                            out=ps_gs[j][:],
                            lhsT=mall[:, gidx * P:(gidx + 1) * P],
                            rhs=gts[:, gidx, :],
                            start=not seen[j], stop=(gidx == last[j]))
                        seen[j] = True
                    for j in range(CPC):
                        p = k * CPC + j
                        pc = adws[j][1]
                        pv = ps_gs[j][:]
                        rec = sbP.tile([P, 2], F32, tag="rec")
                        nc.vector.reciprocal(
                            out=rec[:pc].rearrange("p (h u) -> p h u", u=1),
                            in_=pv.rearrange("p (h c) -> p h c", h=2)[:pc, :, 32:33])
                        hv = sbP.tile([P, HC], F32, tag="hv")
                        nc.vector.tensor_tensor(
                            out=hv[:pc].rearrange("p (h c) -> p h c", h=2),
                            in0=pv.rearrange("p (h c) -> p h c", h=2)[:pc, :, 0:32],
                            in1=rec[:pc].rearrange("p (h u) -> p h u", u=1)
                                .to_broadcast([pc, 2, 32]),
                            op=OP.mult)
                        te = sbP.tile([P, HC], F32, tag="te")
                        nc.vector.tensor_scalar_min(te[:pc], hv[:pc], 0.0)
                        nc.scalar.activation(te[:pc], te[:pc], AF.Exp)
                        xnt = sbP.tile([P, HC], F16, tag="xnt")
                        nc.vector.tensor_scalar_max(hv[:pc], hv[:pc], 0.0)
                        nc.vector.tensor_tensor(out=xnt[:pc], in0=hv[:pc],
                                                in1=te[:pc], op=OP.add)
                        nc.vector.tensor_scalar_add(xnt[:pc], xnt[:pc], -1.0)
                        if l < 2:
                            xt = dramX.tile([pc, HC], F16, tag="xn")
                            nc.sync.dma_start(out=xt[:], in_=xnt[:pc])
                            xn_next[p] = xt
                        else:
                            pgt = sbP.tile([P, G], F16, tag="pgt")
                            nc.sync.dma_start(
                                out=pgt[:],
                                in_=t_pg.ap()[:, p * G:(p + 1) * G])
                            nc.tensor.matmul(out=ps_pool[:], lhsT=pgt[:pc],
                                             rhs=xnt[:pc],
                                             start=(p == 0),
                                             stop=(p == NPAIR - 1))
                xn_tiles = xn_next
            # ---- pool tail + MLP ----
            pool_s = sbT.tile([G, HC], F32, tag="pool_s")
            nc.scalar.copy(pool_s[:], ps_pool[:])
            pool_l = dramP.tile([G, HC], F32, tag="pool_l")
            nc.sync.dma_start(out=pool_l[:], in_=pool_s[:])
            pool_r = dramP.tile([G, HC], F32, tag="pool_r")
            nc.gpsimd.collective_compute(
                "AllReduce", mybir.AluOpType.add,
                replica_groups=[list(range(NCORE))],
                ins=[pool_l.opt()], outs=[pool_r.opt()])
            pooled = sbT.tile([G, HC], F32, tag="pooled")
            nc.sync.dma_start(out=pooled[:], in_=pool_r[:])
            cntt = sbT.tile([G, 1], F32, tag="cntt")
            nc.sync.dma_start(out=cntt[:], in_=t_cnt.ap())
            nc.vector.tensor_scalar_max(cntt[:], cntt[:], 1.0)
            rcc = sbT.tile([G, 1], F32, tag="rcc")
            nc.vector.reciprocal(out=rcc[:], in_=cntt[:])
            nc.vector.tensor_tensor(out=pooled[:], in0=pooled[:],
                                    in1=rcc[:].to_broadcast([G, HC]),
                                    op=mybir.AluOpType.mult)
            pooh = sbT.tile([G, HC], F16, tag="pooh")
            nc.vector.tensor_copy(pooh[:], pooled[:])
            ps_pt = psT.tile([HC, G], F16, space="PSUM", tag="ps_t")
            nc.tensor.transpose(out=ps_pt[:], in_=pooh[:],
                                identity=id16[0:G, 0:G])
            poolT = sbT.tile([HC, G], F16, tag="poolT")
            nc.scalar.copy(poolT[:], ps_pt[:])
            w1 = sbT.tile([HC, 32], F16, tag="w1")
            nc.sync.dma_start(out=w1[:], in_=t_w1T.ap())
            ps_z = psT.tile([G, 32], F32, space="PSUM", tag="ps_r")
            nc.tensor.matmul(out=ps_z[:], lhsT=poolT[:], rhs=w1[:],
                             start=True, stop=True)
            b1t = sbT.tile([G, 32], F32, tag="b1t")
            nc.sync.dma_start(out=b1t[:], in_=t_b1.ap())
            z1 = sbT.tile([G, 32], F32, tag="z1")
            nc.vector.tensor_tensor(out=z1[:], in0=ps_z[:], in1=b1t[:],
                                    op=mybir.AluOpType.add)
            nc.vector.tensor_scalar_max(z1[:], z1[:], 0.0)
            z1h = sbT.tile([G, 32], F16, tag="z1h")
            nc.vector.tensor_copy(z1h[:], z1[:])
            ps_zt = psT.tile([32, G], F16, space="PSUM", tag="ps_t")
            nc.tensor.transpose(out=ps_zt[:], in_=z1h[:],
                                identity=id16[0:G, 0:G])
            z1T = sbT.tile([32, G], F16, tag="z1T")
            nc.scalar.copy(z1T[:], ps_zt[:])
            w2 = sbT.tile([32, 2], F16, tag="w2")
            nc.sync.dma_start(out=w2[:], in_=t_w2T.ap())
            ps_o = psT.tile([G, 2], F32, space="PSUM", tag="ps_r")
            nc.tensor.matmul(out=ps_o[:], lhsT=z1T[:], rhs=w2[:],
                             start=True, stop=True)
            b2t = sbT.tile([G, 2], F32, tag="b2t")
            nc.sync.dma_start(out=b2t[:], in_=t_b2.ap())
            outt = sbT.tile([G, 2], F32, tag="outt")
            nc.vector.tensor_tensor(out=outt[:], in0=ps_o[:], in1=b2t[:],
                                    op=mybir.AluOpType.add)
            nc.sync.dma_start(out=t_out.ap(), in_=outt[:])
    nc.compile()
    return nc


_CACHE = {}


def kernel(**inputs):
    cores, shared = prep(inputs)
    key = (shared['NG'],)
    if key not in _CACHE:
        _CACHE[key] = build(shared)
    nc = _CACHE[key]
    names = ['wcat0', 'wcat1', 'wcat2', 'iota', 'id16', 'counts', 'w1T',
             'w2T', 'b1rep', 'b2rep']
    ins = []
    for c in range(NCORE):
        m = dict(cores[c])
        m.pop('x16', None)
        for n in names:
            m[n] = shared[n]
        ins.append(m)
    res = run_bass_kernel_spmd(nc, ins, core_ids=list(range(NCORE)))
    return np.asarray(res.results[0]['out'], np.float32)
